# revision 21
# baseline (speedup 1.0000x reference)
"""Bilinear discriminator scores = sigmoid((x @ W.T) @ y.T) on 8 NeuronCores.

Sharding: rows of x (and of the output) split 8 ways; y and W replicated.
Per core: xt.T = W.T.T @ x.T via TensorE (K=d), then for each 128-row block
scores = sigmoid(xt.T.T @ y.T) with the contraction over d' on partitions.
All matmuls run in float32r (tf32-like, full-rate); sigmoid on ScalarE
straight out of PSUM with an fp16 store (halves output HBM traffic; sigmoid
outputs live in (0,1) where fp16 keeps ~1e-4 relative error), staged in SBUF
as full [128, 8192] row-blocks for 2 MiB DMA stores, upcast on host.

Host-side prep is layout only: transposes (x.T, y.T, W.T) and row-sharding.
"""

import numpy as np

import concourse.bass as bass
import concourse.tile as tile
from concourse import mybir
from concourse.bass_utils import run_bass_kernel_spmd

N, M, D = 8192, 8192, 256
NCORES = 8
NS = N // NCORES  # 1024 output rows per core
P = 128  # partitions
JBLK = 512  # matmul moving free dim (one PSUM bank of fp32)
JGRP = 2048  # sigmoid batch: 4 PSUM banks per ScalarE activation

_F32 = mybir.dt.float32
_F32R = mybir.dt.float32r
_F16 = mybir.dt.float16


def _split_multi_waits(nc):
    """This walrus build rejects >1 sync-wait per instruction; Tile emits
    several. Move extra waits onto same-engine NoOps inserted just before
    the instruction (same engine stream order => identical semantics)."""
    ctr = 0
    for func in nc.m.functions:
        for bb in func.blocks:
            out = []
            changed = False
            for inst in bb.instructions:
                si = getattr(inst, "sync_info", None)
                waits = list(si.on_wait) if si is not None and si.on_wait else []
                if len(waits) > 1:
                    changed = True
                    for w in waits[:-1]:
                        ctr += 1
                        out.append(
                            mybir.InstNoOp(
                                name=f"split_wait_nop_{ctr}",
                                sync_info=mybir.SyncInfo(on_wait=[w], on_update=[]),
                                bass_nofuse=True,
                                engine=inst.engine,
                            )
                        )
                    si.on_wait = [waits[-1]]
                out.append(inst)
            if changed:
                bb.instructions = out


def _dedup_ldweights(nc):
    """Tile lowers every matmul into an InstLdweights+InstMatmult pair, and
    this walrus build has ldw-opt disabled, so runs of matmuls that share one
    stationary weight set still reload it each time. Stationary weights
    persist in the PE array, so an InstLdweights whose weights AP is
    identical to the previous one (with only matmuls in between) is a no-op:
    delete it and push its sync onto the following instruction."""
    removed = 0
    for func in nc.m.functions:
        for bb in func.blocks:
            insts = list(bb.instructions)
            out = []
            last_ldw_key = None
            pending_sync = []
            changed = False
            for inst in insts:
                t = type(inst).__name__
                if t == "InstLdweights":
                    key = str(inst.ins[0])
                    if key == last_ldw_key:
                        si = getattr(inst, "sync_info", None)
                        if si is not None:
                            pending_sync.append(si)
                        removed += 1
                        changed = True
                        continue
                    last_ldw_key = key
                elif t != "InstMatmult":
                    # any other PE-stream instruction: be conservative
                    if str(inst.engine).endswith("PE"):
                        last_ldw_key = None
                if pending_sync:
                    si = getattr(inst, "sync_info", None)
                    waits = list(si.on_wait) if si is not None and si.on_wait else []
                    ups = list(si.on_update) if si is not None and si.on_update else []
                    for ps in pending_sync:
                        if ps.on_wait:
                            waits.extend(ps.on_wait)
                        if ps.on_update:
                            ups.extend(ps.on_update)
                    inst.sync_info = mybir.SyncInfo(on_wait=waits, on_update=ups)
                    pending_sync = []
                out.append(inst)
            assert not pending_sync
            if changed:
                bb.instructions = out
    return removed


def _emit(nc, tc, xT_ap, yT_ap, wT_ap, out_ap, repeats=1, mm_order="dp", dt2=_F32R):
    import contextlib

    ctx = contextlib.ExitStack()
    with ctx:
        const = ctx.enter_context(tc.tile_pool(name="const", bufs=1))
        outp = ctx.enter_context(tc.tile_pool(name="outp", bufs=3))
        psum = ctx.enter_context(tc.tile_pool(name="psum", bufs=2, space="PSUM"))

        # ---- load inputs (d resp. d' on partitions). Small step-1 inputs
        # first so step 1 runs under the y load; y split into per-j-group
        # chunk tiles so step 2's early groups start before the tail lands.
        wT_sb = []
        xT_sb = []
        for dk in range(2):
            tw = const.tile([P, D], _F32R, name=f"w{dk}", tag=f"w{dk}")
            nc.sync.dma_start(tw[:], wT_ap[dk * P : (dk + 1) * P, :])
            wT_sb.append(tw)
            tx = const.tile([P, NS], _F32R, name=f"x{dk}", tag=f"x{dk}")
            nc.sync.dma_start(tx[:], xT_ap[dk * P : (dk + 1) * P, :])
            xT_sb.append(tx)
        # yT_sb[dk][g]: [P, JGRP] chunk for columns [g*JGRP, (g+1)*JGRP)
        yT_sb = [[None] * (M // JGRP) for _ in range(2)]
        for g in range(M // JGRP):
            for dk in range(2):
                ty = const.tile([P, JGRP], dt2, name=f"y{dk}_{g}", tag=f"y{dk}_{g}")
                nc.sync.dma_start(
                    ty[:], yT_ap[dk * P : (dk + 1) * P, g * JGRP : (g + 1) * JGRP]
                )
                yT_sb[dk][g] = ty

        # ---- prime the ScalarE sigmoid table during the load phase (the
        # ~2.7us ACT_TABLE_LOAD otherwise runs before the first real
        # ACTIVATE on the critical path) ----
        prime = const.tile([P, JBLK], _F32, name="prime", tag="prime")
        nc.gpsimd.memset(prime[:, 0 : JBLK // 2], 0.0)
        nc.scalar.activation(
            prime[:, JBLK // 2 : JBLK],
            prime[:, 0 : JBLK // 2],
            mybir.ActivationFunctionType.Sigmoid,
        )

        # ---- PE pre-warm: dummy matmuls on the (tiny, early-arriving) W
        # tile keep the PE busy during the y load so HAM un-throttles
        # (1.2 -> 2.4 GHz) before the real matmul stream starts. ----
        wps = psum.tile([P, JBLK], _F32, name="wps", tag="ps")
        for _ in range(16):
            nc.tensor.matmul(
                wps[:, 0:D],
                wT_sb[0][:, 0:P],
                wT_sb[0][:, 0:D],
                start=True,
                stop=True,
            )

        # ---- step 1: xtT[d', i] = sum_d W.T[d, d'] * xT[d, i] ----
        xtT_sb = [const.tile([P, NS], dt2, name=f"xt{dp}", tag=f"xt{dp}") for dp in range(2)]
        for _rep in range(repeats):
          for dp in range(2):
            for ic2 in range(NS // JBLK):
                ps = psum.tile([P, JBLK], _F32, name="ps1", tag="ps")
                for dk in range(2):
                    nc.tensor.matmul(
                        ps[:],
                        wT_sb[dk][:, dp * P : (dp + 1) * P],
                        xT_sb[dk][:, ic2 * JBLK : (ic2 + 1) * JBLK],
                        start=(dk == 0),
                        stop=(dk == 1),
                    )
                nc.vector.tensor_copy(
                    xtT_sb[dp][:, ic2 * JBLK : (ic2 + 1) * JBLK], ps[:]
                )

          # ---- step 2: per 128-row block, scores then sigmoid then store.
          # The last block stores per j-group so the kernel tail only waits
          # on a 512 KiB store instead of a full 2 MiB row-block. ----
          for ic in range(NS // P):
            last = ic == NS // P - 1
            ob = outp.tile([P, M], _F16, name="ob", tag="ob")
            for jg in range(M // JGRP):
                ps = psum.tile([P, JGRP], _F32, name="ps2", tag="ps")
                # mm_order="dp": 4 consecutive matmuls share one stationary
                # weight set (the PE re-loads weights per matmul with
                # ldw-opt off). mm_order="js": accumulate each slice
                # immediately (weights alternate every matmul).
                if mm_order == "dp":
                    for dp in range(2):
                        for js in range(JGRP // JBLK):
                            nc.tensor.matmul(
                                ps[:, js * JBLK : (js + 1) * JBLK],
                                xtT_sb[dp][:, ic * P : (ic + 1) * P],
                                yT_sb[dp][jg][:, js * JBLK : (js + 1) * JBLK],
                                start=(dp == 0),
                                stop=(dp == 1),
                            )
                else:
                    for js in range(JGRP // JBLK):
                        for dp in range(2):
                            nc.tensor.matmul(
                                ps[:, js * JBLK : (js + 1) * JBLK],
                                xtT_sb[dp][:, ic * P : (ic + 1) * P],
                                yT_sb[dp][jg][:, js * JBLK : (js + 1) * JBLK],
                                start=(dp == 0),
                                stop=(dp == 1),
                            )
                nc.scalar.activation(
                    ob[:, jg * JGRP : (jg + 1) * JGRP],
                    ps[:],
                    mybir.ActivationFunctionType.Sigmoid,
                )
                if last:
                    nc.sync.dma_start(
                        out_ap[ic * P : (ic + 1) * P, jg * JGRP : (jg + 1) * JGRP],
                        ob[:, jg * JGRP : (jg + 1) * JGRP],
                    )
            if not last:
                nc.sync.dma_start(out_ap[ic * P : (ic + 1) * P, :], ob[:])


_built = {}


def _build(repeats=1, mm_order="dp", dt2_name="f16", dedup=True):
    key = (repeats, mm_order, dt2_name, dedup)
    if key in _built:
        return _built[key]
    dt2 = _F16 if dt2_name == "f16" else _F32R
    nc = bass.Bass("TRN2", target_bir_lowering=False, debug=False, num_devices=NCORES)
    xT_ap = nc.dram_tensor("xT", [D, NS], _F32R, kind="ExternalInput").ap()
    yT_ap = nc.dram_tensor("yT", [D, M], dt2, kind="ExternalInput").ap()
    wT_ap = nc.dram_tensor("wT", [D, D], _F32R, kind="ExternalInput").ap()
    out_ap = nc.dram_tensor("out", [NS, M], _F16, kind="ExternalOutput").ap()
    with tile.TileContext(nc) as tc:
        _emit(nc, tc, xT_ap, yT_ap, wT_ap, out_ap, repeats=repeats, mm_order=mm_order, dt2=dt2)
    if dedup:
        _dedup_ldweights(nc)
    _split_multi_waits(nc)
    _built[key] = nc
    return nc


DT2 = "f16"  # step-2 matmul operand dtype ("f16" or "f32r")


def kernel(x, y, W, **_unused):
    assert x.shape == (N, D) and y.shape == (M, D) and W.shape == (D, D)
    nc = _build(dt2_name=DT2)

    xT = np.ascontiguousarray(x.T.astype(np.float32, copy=False))
    ydt = np.float16 if DT2 == "f16" else np.float32
    yT = np.ascontiguousarray(y.T.astype(ydt))
    wT = np.ascontiguousarray(W.T.astype(np.float32, copy=False))

    in_maps = [
        {
            "xT": np.ascontiguousarray(xT[:, c * NS : (c + 1) * NS]),
            "yT": yT,
            "wT": wT,
        }
        for c in range(NCORES)
    ]
    res = run_bass_kernel_spmd(nc, in_maps, list(range(NCORES))).results
    out = np.empty((N, M), dtype=np.float32)
    for c in range(NCORES):
        out[c * NS : (c + 1) * NS, :] = res[c]["out"]
    return out


# revision 23
# speedup vs baseline: 1.0082x; 1.0082x over previous
"""Bilinear discriminator scores = sigmoid((x @ W.T) @ y.T) on 8 NeuronCores.

Sharding: rows of x (and of the output) split 8 ways; y and W replicated.
Per core: xt.T = W.T.T @ x.T via TensorE (K=d), then for each 128-row block
scores = sigmoid(xt.T.T @ y.T) with the contraction over d' on partitions.
All matmuls run in float32r (tf32-like, full-rate); sigmoid on ScalarE
straight out of PSUM with an fp16 store (halves output HBM traffic; sigmoid
outputs live in (0,1) where fp16 keeps ~1e-4 relative error), staged in SBUF
as full [128, 8192] row-blocks for 2 MiB DMA stores, upcast on host.

Host-side prep is layout only: transposes (x.T, y.T, W.T) and row-sharding.
"""

import numpy as np

import concourse.bass as bass
import concourse.tile as tile
from concourse import mybir
from concourse.bass_utils import run_bass_kernel_spmd

N, M, D = 8192, 8192, 256
NCORES = 8
NS = N // NCORES  # 1024 output rows per core
P = 128  # partitions
JBLK = 512  # matmul moving free dim (one PSUM bank of fp32)
JGRP = 2048  # sigmoid batch: 4 PSUM banks per ScalarE activation

_F32 = mybir.dt.float32
_F32R = mybir.dt.float32r
_F16 = mybir.dt.float16


def _split_multi_waits(nc):
    """This walrus build rejects >1 sync-wait per instruction; Tile emits
    several. Move extra waits onto same-engine NoOps inserted just before
    the instruction (same engine stream order => identical semantics)."""
    ctr = 0
    for func in nc.m.functions:
        for bb in func.blocks:
            out = []
            changed = False
            for inst in bb.instructions:
                si = getattr(inst, "sync_info", None)
                waits = list(si.on_wait) if si is not None and si.on_wait else []
                if len(waits) > 1:
                    changed = True
                    for w in waits[:-1]:
                        ctr += 1
                        out.append(
                            mybir.InstNoOp(
                                name=f"split_wait_nop_{ctr}",
                                sync_info=mybir.SyncInfo(on_wait=[w], on_update=[]),
                                bass_nofuse=True,
                                engine=inst.engine,
                            )
                        )
                    si.on_wait = [waits[-1]]
                out.append(inst)
            if changed:
                bb.instructions = out


def _dedup_ldweights(nc):
    """Tile lowers every matmul into an InstLdweights+InstMatmult pair, and
    this walrus build has ldw-opt disabled, so runs of matmuls that share one
    stationary weight set still reload it each time. Stationary weights
    persist in the PE array, so an InstLdweights whose weights AP is
    identical to the previous one (with only matmuls in between) is a no-op:
    delete it and push its sync onto the following instruction."""
    removed = 0
    for func in nc.m.functions:
        for bb in func.blocks:
            insts = list(bb.instructions)
            out = []
            last_ldw_key = None
            pending_sync = []
            changed = False
            for inst in insts:
                t = type(inst).__name__
                if t == "InstLdweights":
                    key = str(inst.ins[0])
                    if key == last_ldw_key:
                        si = getattr(inst, "sync_info", None)
                        if si is not None:
                            pending_sync.append(si)
                        removed += 1
                        changed = True
                        continue
                    last_ldw_key = key
                elif t != "InstMatmult":
                    # any other PE-stream instruction: be conservative
                    if str(inst.engine).endswith("PE"):
                        last_ldw_key = None
                if pending_sync:
                    si = getattr(inst, "sync_info", None)
                    waits = list(si.on_wait) if si is not None and si.on_wait else []
                    ups = list(si.on_update) if si is not None and si.on_update else []
                    for ps in pending_sync:
                        if ps.on_wait:
                            waits.extend(ps.on_wait)
                        if ps.on_update:
                            ups.extend(ps.on_update)
                    inst.sync_info = mybir.SyncInfo(on_wait=waits, on_update=ups)
                    pending_sync = []
                out.append(inst)
            assert not pending_sync
            if changed:
                bb.instructions = out
    return removed


def _emit(nc, tc, xT_ap, yT_ap, wT_ap, out_ap, repeats=1, mm_order="dp", dt2=_F32R):
    import contextlib

    ctx = contextlib.ExitStack()
    with ctx:
        const = ctx.enter_context(tc.tile_pool(name="const", bufs=1))
        outp = ctx.enter_context(tc.tile_pool(name="outp", bufs=3))
        psum = ctx.enter_context(tc.tile_pool(name="psum", bufs=2, space="PSUM"))

        # ---- load inputs (d resp. d' on partitions). Small step-1 inputs
        # first so step 1 runs under the y load; y split into per-j-group
        # chunk tiles so step 2's early groups start before the tail lands.
        wT_sb = []
        xT_sb = []
        for dk in range(2):
            tw = const.tile([P, D], _F32R, name=f"w{dk}", tag=f"w{dk}")
            nc.sync.dma_start(tw[:], wT_ap[dk * P : (dk + 1) * P, :])
            wT_sb.append(tw)
            tx = const.tile([P, NS], _F32R, name=f"x{dk}", tag=f"x{dk}")
            nc.sync.dma_start(tx[:], xT_ap[dk * P : (dk + 1) * P, :])
            xT_sb.append(tx)
        # yT_sb[dk][g]: [P, YCH] chunk for columns [g*YCH, (g+1)*YCH).
        # Chunks are half a sigmoid group so the first matmul group's
        # operands land sooner during the fill phase.
        YCH = JGRP // 2
        yT_sb = [[None] * (M // YCH) for _ in range(2)]
        for g in range(M // YCH):
            for dk in range(2):
                ty = const.tile([P, YCH], dt2, name=f"y{dk}_{g}", tag=f"y{dk}_{g}")
                nc.sync.dma_start(
                    ty[:], yT_ap[dk * P : (dk + 1) * P, g * YCH : (g + 1) * YCH]
                )
                yT_sb[dk][g] = ty

        # ---- prime the ScalarE sigmoid table during the load phase (the
        # ~2.7us ACT_TABLE_LOAD otherwise runs before the first real
        # ACTIVATE on the critical path) ----
        prime = const.tile([P, JBLK], _F32, name="prime", tag="prime")
        nc.gpsimd.memset(prime[:, 0 : JBLK // 2], 0.0)
        nc.scalar.activation(
            prime[:, JBLK // 2 : JBLK],
            prime[:, 0 : JBLK // 2],
            mybir.ActivationFunctionType.Sigmoid,
        )

        # ---- PE pre-warm: dummy matmuls on the (tiny, early-arriving) W
        # tile keep the PE busy during the y load so HAM un-throttles
        # (1.2 -> 2.4 GHz) before the real matmul stream starts. ----
        wps = psum.tile([P, JBLK], _F32, name="wps", tag="ps")
        for _ in range(16):
            nc.tensor.matmul(
                wps[:, 0:D],
                wT_sb[0][:, 0:P],
                wT_sb[0][:, 0:D],
                start=True,
                stop=True,
            )

        # ---- step 1: xtT[d', i] = sum_d W.T[d, d'] * xT[d, i] ----
        xtT_sb = [const.tile([P, NS], dt2, name=f"xt{dp}", tag=f"xt{dp}") for dp in range(2)]
        for _rep in range(repeats):
          # ic2-outer: the low-i xtT chunks (what step 2's first row-blocks
          # read) are produced first.
          for ic2 in range(NS // JBLK):
            for dp in range(2):
                ps = psum.tile([P, JBLK], _F32, name="ps1", tag="ps")
                for dk in range(2):
                    nc.tensor.matmul(
                        ps[:],
                        wT_sb[dk][:, dp * P : (dp + 1) * P],
                        xT_sb[dk][:, ic2 * JBLK : (ic2 + 1) * JBLK],
                        start=(dk == 0),
                        stop=(dk == 1),
                    )
                nc.vector.tensor_copy(
                    xtT_sb[dp][:, ic2 * JBLK : (ic2 + 1) * JBLK], ps[:]
                )

          # ---- step 2: per 128-row block, scores then sigmoid then store.
          # The last block stores per j-group so the kernel tail only waits
          # on a 512 KiB store instead of a full 2 MiB row-block. ----
          for ic in range(NS // P):
            last = ic == NS // P - 1
            ob = outp.tile([P, M], _F16, name="ob", tag="ob")
            for jg in range(M // JGRP):
                ps = psum.tile([P, JGRP], _F32, name="ps2", tag="ps")
                # mm_order="dp": 4 consecutive matmuls share one stationary
                # weight set (the PE re-loads weights per matmul with
                # ldw-opt off). mm_order="js": accumulate each slice
                # immediately (weights alternate every matmul).
                def _yslice(dp, js):
                    col = jg * JGRP + js * JBLK
                    t = yT_sb[dp][col // (JGRP // 2)]
                    o = col % (JGRP // 2)
                    return t[:, o : o + JBLK]

                if mm_order == "dp":
                    for dp in range(2):
                        for js in range(JGRP // JBLK):
                            nc.tensor.matmul(
                                ps[:, js * JBLK : (js + 1) * JBLK],
                                xtT_sb[dp][:, ic * P : (ic + 1) * P],
                                _yslice(dp, js),
                                start=(dp == 0),
                                stop=(dp == 1),
                            )
                else:
                    for js in range(JGRP // JBLK):
                        for dp in range(2):
                            nc.tensor.matmul(
                                ps[:, js * JBLK : (js + 1) * JBLK],
                                xtT_sb[dp][:, ic * P : (ic + 1) * P],
                                _yslice(dp, js),
                                start=(dp == 0),
                                stop=(dp == 1),
                            )
                nc.scalar.activation(
                    ob[:, jg * JGRP : (jg + 1) * JGRP],
                    ps[:],
                    mybir.ActivationFunctionType.Sigmoid,
                )
                if last:
                    nc.sync.dma_start(
                        out_ap[ic * P : (ic + 1) * P, jg * JGRP : (jg + 1) * JGRP],
                        ob[:, jg * JGRP : (jg + 1) * JGRP],
                    )
            if not last:
                nc.sync.dma_start(out_ap[ic * P : (ic + 1) * P, :], ob[:])


_built = {}


def _build(repeats=1, mm_order="dp", dt2_name="f16", dedup=True):
    key = (repeats, mm_order, dt2_name, dedup)
    if key in _built:
        return _built[key]
    dt2 = _F16 if dt2_name == "f16" else _F32R
    nc = bass.Bass("TRN2", target_bir_lowering=False, debug=False, num_devices=NCORES)
    xT_ap = nc.dram_tensor("xT", [D, NS], _F32R, kind="ExternalInput").ap()
    yT_ap = nc.dram_tensor("yT", [D, M], dt2, kind="ExternalInput").ap()
    wT_ap = nc.dram_tensor("wT", [D, D], _F32R, kind="ExternalInput").ap()
    out_ap = nc.dram_tensor("out", [NS, M], _F16, kind="ExternalOutput").ap()
    with tile.TileContext(nc) as tc:
        _emit(nc, tc, xT_ap, yT_ap, wT_ap, out_ap, repeats=repeats, mm_order=mm_order, dt2=dt2)
    if dedup:
        _dedup_ldweights(nc)
    _split_multi_waits(nc)
    _built[key] = nc
    return nc


DT2 = "f16"  # step-2 matmul operand dtype ("f16" or "f32r")


def kernel(x, y, W, **_unused):
    assert x.shape == (N, D) and y.shape == (M, D) and W.shape == (D, D)
    nc = _build(dt2_name=DT2)

    xT = np.ascontiguousarray(x.T.astype(np.float32, copy=False))
    ydt = np.float16 if DT2 == "f16" else np.float32
    yT = np.ascontiguousarray(y.T.astype(ydt))
    wT = np.ascontiguousarray(W.T.astype(np.float32, copy=False))

    in_maps = [
        {
            "xT": np.ascontiguousarray(xT[:, c * NS : (c + 1) * NS]),
            "yT": yT,
            "wT": wT,
        }
        for c in range(NCORES)
    ]
    res = run_bass_kernel_spmd(nc, in_maps, list(range(NCORES))).results
    out = np.empty((N, M), dtype=np.float32)
    for c in range(NCORES):
        out[c * NS : (c + 1) * NS, :] = res[c]["out"]
    return out


# revision 25
# speedup vs baseline: 1.1077x; 1.0987x over previous
"""Bilinear discriminator scores = sigmoid((x @ W.T) @ y.T) on 8 NeuronCores.

Sharding: rows of x (and of the output) split 8 ways; y and W replicated.
Per core: xt.T = W.T.T @ x.T via TensorE (K=d), then for each 128-row block
scores = sigmoid(xt.T.T @ y.T) with the contraction over d' on partitions.
All matmuls run in float32r (tf32-like, full-rate); sigmoid on ScalarE
straight out of PSUM with an fp16 store (halves output HBM traffic; sigmoid
outputs live in (0,1) where fp16 keeps ~1e-4 relative error), staged in SBUF
as full [128, 8192] row-blocks for 2 MiB DMA stores, upcast on host.

Host-side prep is layout only: transposes (x.T, y.T, W.T) and row-sharding.
"""

import numpy as np

import concourse.bass as bass
import concourse.tile as tile
from concourse import mybir
from concourse.bass_utils import run_bass_kernel_spmd

N, M, D = 8192, 8192, 256
NCORES = 8
NS = N // NCORES  # 1024 output rows per core
P = 128  # partitions
JBLK = 512  # matmul moving free dim (one PSUM bank of fp32)
JGRP = 2048  # sigmoid batch: 4 PSUM banks per ScalarE activation

_F32 = mybir.dt.float32
_F32R = mybir.dt.float32r
_F16 = mybir.dt.float16


def _split_multi_waits(nc):
    """This walrus build rejects >1 sync-wait per instruction; Tile emits
    several. Move extra waits onto same-engine NoOps inserted just before
    the instruction (same engine stream order => identical semantics)."""
    ctr = 0
    for func in nc.m.functions:
        for bb in func.blocks:
            out = []
            changed = False
            for inst in bb.instructions:
                si = getattr(inst, "sync_info", None)
                waits = list(si.on_wait) if si is not None and si.on_wait else []
                if len(waits) > 1:
                    changed = True
                    for w in waits[:-1]:
                        ctr += 1
                        out.append(
                            mybir.InstNoOp(
                                name=f"split_wait_nop_{ctr}",
                                sync_info=mybir.SyncInfo(on_wait=[w], on_update=[]),
                                bass_nofuse=True,
                                engine=inst.engine,
                            )
                        )
                    si.on_wait = [waits[-1]]
                out.append(inst)
            if changed:
                bb.instructions = out


def _dedup_ldweights(nc):
    """Tile lowers every matmul into an InstLdweights+InstMatmult pair, and
    this walrus build has ldw-opt disabled, so runs of matmuls that share one
    stationary weight set still reload it each time. Stationary weights
    persist in the PE array, so an InstLdweights whose weights AP is
    identical to the previous one (with only matmuls in between) is a no-op:
    delete it and push its sync onto the following instruction."""
    removed = 0
    for func in nc.m.functions:
        for bb in func.blocks:
            insts = list(bb.instructions)
            out = []
            last_ldw_key = None
            pending_sync = []
            changed = False
            for inst in insts:
                t = type(inst).__name__
                if t == "InstLdweights":
                    key = str(inst.ins[0])
                    if key == last_ldw_key:
                        si = getattr(inst, "sync_info", None)
                        if si is not None:
                            pending_sync.append(si)
                        removed += 1
                        changed = True
                        continue
                    last_ldw_key = key
                elif t != "InstMatmult":
                    # any other PE-stream instruction: be conservative
                    if str(inst.engine).endswith("PE"):
                        last_ldw_key = None
                if pending_sync:
                    si = getattr(inst, "sync_info", None)
                    waits = list(si.on_wait) if si is not None and si.on_wait else []
                    ups = list(si.on_update) if si is not None and si.on_update else []
                    for ps in pending_sync:
                        if ps.on_wait:
                            waits.extend(ps.on_wait)
                        if ps.on_update:
                            ups.extend(ps.on_update)
                    inst.sync_info = mybir.SyncInfo(on_wait=waits, on_update=ups)
                    pending_sync = []
                out.append(inst)
            assert not pending_sync
            if changed:
                bb.instructions = out
    return removed


def _thin_pe_updates(nc):
    """Every lowered matmul carries a +1 update on the PE proc sem (~26ns
    each, serialized on the EVT_SEM register), but consumers only wait on
    group-final ticks. Drop un-waited updates and renumber the waits."""
    # PE-stream updates in program order, and all waits, per sem id
    upds = {}   # sem_id -> list of (inst, update_entry_index, cum_tick)
    waits = {}  # sem_id -> list of (sync_info, wait_index, value)
    for func in nc.m.functions:
        for bb in func.blocks:
            for inst in bb.instructions:
                si = getattr(inst, "sync_info", None)
                if si is None:
                    continue
                if (
                    type(inst).__name__ == "InstMatmult"
                    and str(inst.engine).endswith("PE")
                    and si.on_update
                ):
                    for ui, u in enumerate(si.on_update):
                        if u.update_mode == "sem-inc" and u.update_value == 1:
                            lst = upds.setdefault(u.id, [])
                            lst.append((inst, ui, len(lst) + 1))
                if si.on_wait:
                    for wi, w in enumerate(si.on_wait):
                        if w.wait_mode == "sem-ge-imm":
                            waits.setdefault(w.id, []).append((si, wi, w.wait_value))
    for sem_id, ulist in upds.items():
        if len(ulist) < 8:
            continue
        waited = {v for (_, _, v) in waits.get(sem_id, [])}
        keep_ticks = sorted(waited | {ulist[-1][2]})
        if not all(1 <= t <= len(ulist) for t in keep_ticks):
            continue  # something references ticks we don't model; skip
        kept = set(keep_ticks)
        for inst, ui, tick in ulist:
            if tick not in kept:
                si = inst.sync_info
                ups = list(si.on_update)
                ups.pop(ui)
                inst.sync_info = mybir.SyncInfo(
                    on_wait=list(si.on_wait) if si.on_wait else [], on_update=ups
                )
        import bisect
        for si, wi, v in waits.get(sem_id, []):
            ws = list(si.on_wait)
            w = ws[wi]
            new_v = bisect.bisect_right(keep_ticks, v)
            # v is in keep_ticks, so new_v = its 1-based rank
            w.wait_value = new_v
            ws[wi] = w
            si.on_wait = ws


def _emit(nc, tc, xT_ap, yT_ap, wT_ap, out_ap, repeats=1, mm_order="dp", dt2=_F32R):
    import contextlib

    ctx = contextlib.ExitStack()
    with ctx:
        const = ctx.enter_context(tc.tile_pool(name="const", bufs=1))
        outp = ctx.enter_context(tc.tile_pool(name="outp", bufs=3))
        psum = ctx.enter_context(tc.tile_pool(name="psum", bufs=2, space="PSUM"))

        # ---- load inputs (d resp. d' on partitions). Small step-1 inputs
        # first so step 1 runs under the y load; y split into per-j-group
        # chunk tiles so step 2's early groups start before the tail lands.
        wT_sb = []
        xT_sb = []
        for dk in range(2):
            tw = const.tile([P, D], _F32R, name=f"w{dk}", tag=f"w{dk}")
            nc.sync.dma_start(tw[:], wT_ap[dk * P : (dk + 1) * P, :])
            wT_sb.append(tw)
            tx = const.tile([P, NS], _F32R, name=f"x{dk}", tag=f"x{dk}")
            nc.sync.dma_start(tx[:], xT_ap[dk * P : (dk + 1) * P, :])
            xT_sb.append(tx)
        # yT_sb[dk][g]: [P, YCH] chunk for columns [g*YCH, (g+1)*YCH).
        # Chunks are half a sigmoid group so the first matmul group's
        # operands land sooner during the fill phase.
        YCH = JGRP // 2
        yT_sb = [[None] * (M // YCH) for _ in range(2)]
        for g in range(M // YCH):
            for dk in range(2):
                ty = const.tile([P, YCH], dt2, name=f"y{dk}_{g}", tag=f"y{dk}_{g}")
                nc.sync.dma_start(
                    ty[:], yT_ap[dk * P : (dk + 1) * P, g * YCH : (g + 1) * YCH]
                )
                yT_sb[dk][g] = ty

        # ---- prime the ScalarE sigmoid table during the load phase (the
        # ~2.7us ACT_TABLE_LOAD otherwise runs before the first real
        # ACTIVATE on the critical path) ----
        prime = const.tile([P, JBLK], _F32, name="prime", tag="prime")
        nc.gpsimd.memset(prime[:, 0 : JBLK // 2], 0.0)
        nc.scalar.activation(
            prime[:, JBLK // 2 : JBLK],
            prime[:, 0 : JBLK // 2],
            mybir.ActivationFunctionType.Sigmoid,
        )

        # ---- PE pre-warm: dummy matmuls on the (tiny, early-arriving) W
        # tile keep the PE busy during the y load so HAM un-throttles
        # (1.2 -> 2.4 GHz) before the real matmul stream starts. ----
        wps = psum.tile([P, JBLK], _F32, name="wps", tag="ps")
        for _ in range(16):
            nc.tensor.matmul(
                wps[:, 0:D],
                wT_sb[0][:, 0:P],
                wT_sb[0][:, 0:D],
                start=True,
                stop=True,
            )

        # ---- step 1: xtT[d', i] = sum_d W.T[d, d'] * xT[d, i] ----
        xtT_sb = [const.tile([P, NS], dt2, name=f"xt{dp}", tag=f"xt{dp}") for dp in range(2)]
        for _rep in range(repeats):
          # ic2-outer: the low-i xtT chunks (what step 2's first row-blocks
          # read) are produced first.
          for ic2 in range(NS // JBLK):
            for dp in range(2):
                ps = psum.tile([P, JBLK], _F32, name="ps1", tag="ps")
                for dk in range(2):
                    nc.tensor.matmul(
                        ps[:],
                        wT_sb[dk][:, dp * P : (dp + 1) * P],
                        xT_sb[dk][:, ic2 * JBLK : (ic2 + 1) * JBLK],
                        start=(dk == 0),
                        stop=(dk == 1),
                    )
                nc.vector.tensor_copy(
                    xtT_sb[dp][:, ic2 * JBLK : (ic2 + 1) * JBLK], ps[:]
                )

          # ---- step 2: per 128-row block, scores then sigmoid then store.
          # The last block stores per j-group so the kernel tail only waits
          # on a 512 KiB store instead of a full 2 MiB row-block. ----
          for ic in range(NS // P):
            last = ic == NS // P - 1
            ob = outp.tile([P, M], _F16, name="ob", tag="ob")
            for jg in range(M // JGRP):
                ps = psum.tile([P, JGRP], _F32, name="ps2", tag="ps")
                # mm_order="dp": 4 consecutive matmuls share one stationary
                # weight set (the PE re-loads weights per matmul with
                # ldw-opt off). mm_order="js": accumulate each slice
                # immediately (weights alternate every matmul).
                def _yslice(dp, js):
                    col = jg * JGRP + js * JBLK
                    t = yT_sb[dp][col // (JGRP // 2)]
                    o = col % (JGRP // 2)
                    return t[:, o : o + JBLK]

                if mm_order == "probe_half":
                    for js in range(JGRP // JBLK):
                        nc.tensor.matmul(
                            ps[:, js * JBLK : (js + 1) * JBLK],
                            xtT_sb[0][:, ic * P : (ic + 1) * P],
                            _yslice(0, js),
                            start=True,
                            stop=True,
                        )
                elif mm_order == "dp":
                    for dp in range(2):
                        for js in range(JGRP // JBLK):
                            nc.tensor.matmul(
                                ps[:, js * JBLK : (js + 1) * JBLK],
                                xtT_sb[dp][:, ic * P : (ic + 1) * P],
                                _yslice(dp, js),
                                start=(dp == 0),
                                stop=(dp == 1),
                            )
                else:
                    for js in range(JGRP // JBLK):
                        for dp in range(2):
                            nc.tensor.matmul(
                                ps[:, js * JBLK : (js + 1) * JBLK],
                                xtT_sb[dp][:, ic * P : (ic + 1) * P],
                                _yslice(dp, js),
                                start=(dp == 0),
                                stop=(dp == 1),
                            )
                nc.scalar.activation(
                    ob[:, jg * JGRP : (jg + 1) * JGRP],
                    ps[:],
                    mybir.ActivationFunctionType.Sigmoid,
                )
                if last:
                    nc.sync.dma_start(
                        out_ap[ic * P : (ic + 1) * P, jg * JGRP : (jg + 1) * JGRP],
                        ob[:, jg * JGRP : (jg + 1) * JGRP],
                    )
            if not last:
                nc.sync.dma_start(out_ap[ic * P : (ic + 1) * P, :], ob[:])


_built = {}


def _build(repeats=1, mm_order="dp", dt2_name="f16", dedup=True):
    key = (repeats, mm_order, dt2_name, dedup)
    if key in _built:
        return _built[key]
    dt2 = _F16 if dt2_name == "f16" else _F32R
    nc = bass.Bass("TRN2", target_bir_lowering=False, debug=False, num_devices=NCORES)
    xT_ap = nc.dram_tensor("xT", [D, NS], _F32R, kind="ExternalInput").ap()
    yT_ap = nc.dram_tensor("yT", [D, M], dt2, kind="ExternalInput").ap()
    wT_ap = nc.dram_tensor("wT", [D, D], _F32R, kind="ExternalInput").ap()
    out_ap = nc.dram_tensor("out", [NS, M], _F16, kind="ExternalOutput").ap()
    with tile.TileContext(nc) as tc:
        _emit(nc, tc, xT_ap, yT_ap, wT_ap, out_ap, repeats=repeats, mm_order=mm_order, dt2=dt2)
    if dedup:
        _dedup_ldweights(nc)
        _thin_pe_updates(nc)
    _split_multi_waits(nc)
    _built[key] = nc
    return nc


DT2 = "f16"  # step-2 matmul operand dtype ("f16" or "f32r")


def kernel(x, y, W, **_unused):
    assert x.shape == (N, D) and y.shape == (M, D) and W.shape == (D, D)
    nc = _build(dt2_name=DT2)

    xT = np.ascontiguousarray(x.T.astype(np.float32, copy=False))
    ydt = np.float16 if DT2 == "f16" else np.float32
    yT = np.ascontiguousarray(y.T.astype(ydt))
    wT = np.ascontiguousarray(W.T.astype(np.float32, copy=False))

    in_maps = [
        {
            "xT": np.ascontiguousarray(xT[:, c * NS : (c + 1) * NS]),
            "yT": yT,
            "wT": wT,
        }
        for c in range(NCORES)
    ]
    res = run_bass_kernel_spmd(nc, in_maps, list(range(NCORES))).results
    out = np.empty((N, M), dtype=np.float32)
    for c in range(NCORES):
        out[c * NS : (c + 1) * NS, :] = res[c]["out"]
    return out
